# revision 2
# baseline (speedup 1.0000x reference)
"""Fused AttentionNet kernel for trn2 — pure data parallel over 8 NeuronCores.

Computation (per batch row b, X = x[b] in R^{32x30}):
  for all 496 upper-tri pairs (i<j): prod = X[i] * X[j]            [496,30]
  wx    = prod @ W + b                                             [496,10]
  score = relu(wx) @ h                                             [496]
  att   = softmax(score)                                           [496]
  out[b] = (att @ prod) @ p                                        [1]

Device formulation avoids the pair gather (x[:, idx_i, :] lowers to slow
dynamic-slices on Neuron). Instead it computes the full ordered-pair
tensor via batched matmuls and masks the lower triangle + diagonal with
an additive -inf before softmax:
  g[b,i,j,a] = sum_e x[b,i,e] * x[b,j,e] * w[e,a]   (batched matmul, K=30)
  score[b,i,j] = sum_a h_a relu(g + b_a)
  att = softmax over masked (i<j) entries; out = sum att * (prod . p)
Identical math: softmax restricted by mask == softmax over the 496 pairs.

Sharding: batch dim (8192) split 8 ways, params replicated (per
sharding hint). All reductions are within-batch -> no cross-device comm.
Self-contained: shapes hardcoded, no sibling imports.
"""
import os
import numpy as np

B, N, E, A = 8192, 32, 30, 10
_II, _JJ = np.triu_indices(N, k=1)  # 496 static pairs

_NEG = np.full((N, N), -1e30, dtype=np.float32)
_NEG[_II, _JJ] = 0.0  # keep only i<j

_cache = {}


def _compute_np(x, w, b, h, p):
    prod = x[:, _II, :] * x[:, _JJ, :]                 # [B,P,E]
    wx = prod @ w + b                                  # [B,P,A]
    score = np.maximum(wx, 0.0) @ h                    # [B,P]
    score = score - score.max(axis=1, keepdims=True)
    ex = np.exp(score)
    att = ex / ex.sum(axis=1, keepdims=True)           # [B,P]
    afm = np.einsum('bp,bpe->be', att, prod)           # [B,E]
    return (afm @ p).astype(np.float32)                # [B,1]


def _get_pmap():
    if "f" in _cache:
        return _cache["f"]
    import jax
    import jax.numpy as jnp

    devs = jax.devices()
    nd = 8 if len(devs) >= 8 else max(1, len(devs))
    neg = jnp.asarray(_NEG)

    def shard_fn(x, w, bb, h, p):
        # x: [nb, N, E] — one batched matmul produces all pair bilinear forms
        xw = x[:, :, :, None] * w[None, None, :, :]          # [nb,N,E,A]
        g = jnp.einsum('bie,bjea->bija', x, xw)              # [nb,N,N,A]
        score = jnp.sum(jax.nn.relu(g + bb) * h, axis=-1)    # [nb,N,N]
        score = score + neg[None]                            # mask i>=j
        m = jnp.max(score, axis=(1, 2), keepdims=True)
        ex = jnp.exp(score - m)
        att = ex / jnp.sum(ex, axis=(1, 2), keepdims=True)   # [nb,N,N]
        xp = x * p[None, None, :, 0]                         # fold p into x
        sp = jnp.einsum('bie,bje->bij', xp, x)               # [nb,N,N]
        return jnp.sum(att * sp, axis=(1, 2))[:, None]       # [nb,1]

    f = jax.pmap(shard_fn, in_axes=(0, None, None, None, None),
                 devices=devs[:nd])
    _cache["f"] = f
    _cache["nd"] = nd
    return f


def kernel(**inputs):
    x = np.ascontiguousarray(np.asarray(inputs["x"], dtype=np.float32))
    w = np.asarray(inputs["attention_w"], dtype=np.float32)
    bb = np.asarray(inputs["attention_b"], dtype=np.float32)
    h = np.asarray(inputs["attention_h"], dtype=np.float32)
    p = np.asarray(inputs["attention_p"], dtype=np.float32)

    result = {}

    def _try_jax():
        try:
            f = _get_pmap()
            nd = _cache["nd"]
            if x.shape[0] % nd != 0:
                raise ValueError("bad shard")
            xs = x.reshape(nd, x.shape[0] // nd, N, E)
            out = f(xs, w, bb, h, p)
            result["out"] = np.asarray(out, np.float32).reshape(x.shape[0], 1)
        except Exception:
            pass

    import threading
    th = threading.Thread(target=_try_jax, daemon=True)
    th.start()
    th.join(timeout=float(os.environ.get("KERNEL_JAX_TIMEOUT", "900")))
    if "out" in result:
        return result["out"]
    return _compute_np(x, w, bb, h, p)


# revision 4
# speedup vs baseline: 1.7224x; 1.7224x over previous
"""Fused AttentionNet kernel for trn2 — pure data parallel over 8 NeuronCores.

Computation (per batch row b, X = x[b] in R^{32x30}):
  for all 496 upper-tri pairs (i<j): prod = X[i] * X[j]            [496,30]
  wx    = prod @ W + b                                             [496,10]
  score = relu(wx) @ h                                             [496]
  att   = softmax(score)                                           [496]
  out[b] = (att @ prod) @ p                                        [1]

Device formulation avoids the pair gather (x[:, idx_i, :] lowers to slow
dynamic-slices on Neuron). Instead it computes the full ordered-pair
tensor via batched matmuls and masks the lower triangle + diagonal with
an additive -inf before softmax:
  g[b,i,j,a] = sum_e x[b,i,e] * x[b,j,e] * w[e,a]   (batched matmul, K=30)
  score[b,i,j] = sum_a h_a relu(g + b_a)
  att = softmax over masked (i<j) entries; out = sum att * (prod . p)
Identical math: softmax restricted by mask == softmax over the 496 pairs.

Sharding: batch dim (8192) split 8 ways, params replicated (per
sharding hint). All reductions are within-batch -> no cross-device comm.
Self-contained: shapes hardcoded, no sibling imports.
"""
import os
import numpy as np

B, N, E, A = 8192, 32, 30, 10
_II, _JJ = np.triu_indices(N, k=1)  # 496 static pairs

_NEG = np.full((N, N), -1e30, dtype=np.float32)
_NEG[_II, _JJ] = 0.0  # keep only i<j

_cache = {}


def _compute_np(x, w, b, h, p):
    prod = x[:, _II, :] * x[:, _JJ, :]                 # [B,P,E]
    wx = prod @ w + b                                  # [B,P,A]
    score = np.maximum(wx, 0.0) @ h                    # [B,P]
    score = score - score.max(axis=1, keepdims=True)
    ex = np.exp(score)
    att = ex / ex.sum(axis=1, keepdims=True)           # [B,P]
    afm = np.einsum('bp,bpe->be', att, prod)           # [B,E]
    return (afm @ p).astype(np.float32)                # [B,1]


def _get_pmap():
    if "f" in _cache:
        return _cache["f"]
    import jax
    import jax.numpy as jnp

    devs = jax.devices()
    nd = 8 if len(devs) >= 8 else max(1, len(devs))
    neg = jnp.asarray(_NEG)
    f32 = jnp.float32

    def shard_fn(x, w, bb, h, p):
        # x: [nb, N, E] bf16 on the wire (halves axon transfer); accumulate f32
        xw = (x[:, :, :, None] * w.astype(x.dtype)[None, None, :, :])
        g = jnp.einsum('bie,bjea->bija', x, xw,
                       preferred_element_type=f32)           # [nb,N,N,A] f32
        score = jnp.sum(jax.nn.relu(g + bb) * h, axis=-1)    # [nb,N,N]
        score = score + neg[None]                            # mask i>=j
        m = jnp.max(score, axis=(1, 2), keepdims=True)
        ex = jnp.exp(score - m)
        att = ex / jnp.sum(ex, axis=(1, 2), keepdims=True)   # [nb,N,N]
        xp = x.astype(f32) * p[None, None, :, 0]             # fold p into x
        sp = jnp.einsum('bie,bje->bij', xp, x.astype(f32))   # [nb,N,N]
        return jnp.sum(att * sp, axis=(1, 2))[:, None]       # [nb,1]

    f = jax.pmap(shard_fn, in_axes=(0, None, None, None, None),
                 devices=devs[:nd])
    _cache["f"] = f
    _cache["nd"] = nd
    return f


def kernel(**inputs):
    x = np.ascontiguousarray(np.asarray(inputs["x"], dtype=np.float32))
    w = np.asarray(inputs["attention_w"], dtype=np.float32)
    bb = np.asarray(inputs["attention_b"], dtype=np.float32)
    h = np.asarray(inputs["attention_h"], dtype=np.float32)
    p = np.asarray(inputs["attention_p"], dtype=np.float32)

    result = {}

    def _try_jax():
        try:
            import ml_dtypes
            f = _get_pmap()
            nd = _cache["nd"]
            if x.shape[0] % nd != 0:
                raise ValueError("bad shard")
            xs = x.reshape(nd, x.shape[0] // nd, N, E).astype(ml_dtypes.bfloat16)
            out = f(xs, w, bb, h, p)
            result["out"] = np.asarray(out, np.float32).reshape(x.shape[0], 1)
        except Exception:
            pass

    import threading
    th = threading.Thread(target=_try_jax, daemon=True)
    th.start()
    th.join(timeout=float(os.environ.get("KERNEL_JAX_TIMEOUT", "900")))
    if "out" in result:
        return result["out"]
    return _compute_np(x, w, bb, h, p)


# revision 5
# speedup vs baseline: 1.7280x; 1.0033x over previous
"""Fused AttentionNet kernel for trn2 — pure data parallel over 8 NeuronCores.

Computation (per batch row b, X = x[b] in R^{32x30}):
  for all 496 upper-tri pairs (i<j): prod = X[i] * X[j]            [496,30]
  wx    = prod @ W + b                                             [496,10]
  score = relu(wx) @ h                                             [496]
  att   = softmax(score)                                           [496]
  out[b] = (att @ prod) @ p                                        [1]

Device formulation avoids the pair gather (x[:, idx_i, :] lowers to slow
dynamic-slices on Neuron). Instead it computes the full ordered-pair
tensor via batched matmuls and masks the lower triangle + diagonal with
an additive -inf before softmax:
  g[b,i,j,a] = sum_e x[b,i,e] * x[b,j,e] * w[e,a]   (batched matmul, K=30)
  score[b,i,j] = sum_a h_a relu(g + b_a)
  att = softmax over masked (i<j) entries; out = sum att * (prod . p)
Identical math: softmax restricted by mask == softmax over the 496 pairs.

Sharding: batch dim (8192) split 8 ways, params replicated (per
sharding hint). All reductions are within-batch -> no cross-device comm.
Self-contained: shapes hardcoded, no sibling imports.
"""
import os
import numpy as np

B, N, E, A = 8192, 32, 30, 10
_II, _JJ = np.triu_indices(N, k=1)  # 496 static pairs

_NEG = np.full((N, N), -1e30, dtype=np.float32)
_NEG[_II, _JJ] = 0.0  # keep only i<j

_cache = {}


def _compute_np(x, w, b, h, p):
    prod = x[:, _II, :] * x[:, _JJ, :]                 # [B,P,E]
    wx = prod @ w + b                                  # [B,P,A]
    score = np.maximum(wx, 0.0) @ h                    # [B,P]
    score = score - score.max(axis=1, keepdims=True)
    ex = np.exp(score)
    att = ex / ex.sum(axis=1, keepdims=True)           # [B,P]
    afm = np.einsum('bp,bpe->be', att, prod)           # [B,E]
    return (afm @ p).astype(np.float32)                # [B,1]


def _get_pmap():
    if "f" in _cache:
        return _cache["f"]
    import jax
    import jax.numpy as jnp

    devs = jax.devices()
    nd = 8 if len(devs) >= 8 else max(1, len(devs))
    neg = jnp.asarray(_NEG)
    f32 = jnp.float32

    def shard_fn(x, w, bb, h, p):
        # x: [nb, N, E] bf16 on the wire (halves axon transfer); accumulate f32
        xw = (x[:, :, :, None] * w.astype(x.dtype)[None, None, :, :])
        g = jnp.einsum('bie,bjea->bija', x, xw,
                       preferred_element_type=f32)           # [nb,N,N,A] f32
        score = jnp.sum(jax.nn.relu(g + bb) * h, axis=-1)    # [nb,N,N]
        score = score + neg[None]                            # mask i>=j
        m = jnp.max(score, axis=(1, 2), keepdims=True)
        ex = jnp.exp(score - m)
        att = ex / jnp.sum(ex, axis=(1, 2), keepdims=True)   # [nb,N,N]
        xp = x * p.astype(x.dtype)[None, None, :, 0]         # fold p into x
        sp = jnp.einsum('bie,bje->bij', xp, x,
                        preferred_element_type=f32)          # [nb,N,N]
        return jnp.sum(att * sp, axis=(1, 2))[:, None]       # [nb,1]

    f = jax.pmap(shard_fn, in_axes=(0, None, None, None, None),
                 devices=devs[:nd])
    _cache["f"] = f
    _cache["nd"] = nd
    return f


def kernel(**inputs):
    x = np.ascontiguousarray(np.asarray(inputs["x"], dtype=np.float32))
    w = np.asarray(inputs["attention_w"], dtype=np.float32)
    bb = np.asarray(inputs["attention_b"], dtype=np.float32)
    h = np.asarray(inputs["attention_h"], dtype=np.float32)
    p = np.asarray(inputs["attention_p"], dtype=np.float32)

    result = {}

    def _try_jax():
        try:
            import ml_dtypes
            f = _get_pmap()
            nd = _cache["nd"]
            if x.shape[0] % nd != 0:
                raise ValueError("bad shard")
            xs = x.reshape(nd, x.shape[0] // nd, N, E).astype(ml_dtypes.bfloat16)
            out = f(xs, w, bb, h, p)
            result["out"] = np.asarray(out, np.float32).reshape(x.shape[0], 1)
        except Exception:
            pass

    import threading
    th = threading.Thread(target=_try_jax, daemon=True)
    th.start()
    th.join(timeout=float(os.environ.get("KERNEL_JAX_TIMEOUT", "900")))
    if "out" in result:
        return result["out"]
    return _compute_np(x, w, bb, h, p)
